# revision 27
# baseline (speedup 1.0000x reference)
"""Trainium2 Bass kernel for per-position head-attention (nn_DariushFlashAttention2).

Math (per batch b, sequence position s):
    Q = q[b,s].reshape(H=32, D=128); K, V likewise
    logits = Q @ K.T / sqrt(D)          # [32, 32] attention over HEADS
    W = softmax(logits, axis=-1)
    out[b,s] = (W @ V).reshape(H*D)

Every one of the B*S = 8192 positions is independent, so we shard positions
across the 8 NeuronCores (1024 positions each) and run one SPMD program.

Device strategy (per core):
  - Positions are packed 4-per-group onto the 128 SBUF partitions
    (partition = 4*32 = pos_in_group x head).  Host pre-transposes q,k into
    [d, (g,i,h)] fp16 and packs v as [(i,gh), (g, d|1)] with a ones-column
    per group; q|k|v are CONCATENATED into one DRAM tensor so each
    64-position chunk needs a single ~1.5 MB input DMA (16 total — the Tile
    framework has only 8 HWDGE DMA semaphore lanes, so fewer/bigger DMAs
    keep the prefetch runway deep).
  - QK: per position one col-tiled matmul (tile_position=(0,32j)) into a
    full-bank [128,512] PSUM tile that collects the whole chunk's logits;
    ONE exp() per chunk on ScalarE ([128,512], ~0.7us).
  - WV: per position a (32j,32j) sub-array matmul whose stationary operand
    is that position's [g,h] exp block; V carries a ones-column per group so
    the same matmul emits the softmax denominator in its last column.
  - PSUM is evacuated with plain wide [128,258] copies (no reciprocal, no
    scaling): unnormalized outputs AND denominators ship to the host, which
    does the final divide in fp32.  This removes all RECIPROCAL work and
    halves the evacuation op count.
  - Software pipeline: QK(n+1) is issued before WV(n) so the in-order
    tensor engine can fill the exp(n) latency with QK work instead of
    parking its 4-deep wait queue on WV.
  - Output DMA triggers ride the Sync ring interleaved after the first 9
    input triggers (out(n) is ready before in(n+9) wants to fire, so
    prefetch is never head-of-line blocked); the scalar stream carries only
    exp + two early copies per chunk.
"""

import numpy as np

B, S, H, D = 2, 4096, 32, 128
NCORES = 8
POS = B * S                  # 8192 positions total
PPC = POS // NCORES          # 1024 positions per core
GP = 4                       # positions per group (4*32 heads = 128 partitions)
NG = 16                      # groups per chunk
CHUNK_POS = GP * NG          # 64 positions per chunk
NCHUNK = PPC // CHUNK_POS    # 16 chunks per core
VCOL = D + 1                 # v columns per group incl. ones column

QK_COLS = NG * D             # 2048
IN_COLS = 2 * QK_COLS + NG * VCOL   # q | k | v  = 6160
V_OFF = 2 * QK_COLS          # 4096
OUT_COLS = NG * VCOL         # 2064 = 8 pairs x 258

_SCALE = float(1.0 / np.sqrt(D))

_program = None  # cached compiled Bass program


def _build_program():
    import concourse.bacc as bacc
    import concourse.mybir as mybir
    from concourse.tile import TileContext

    fp32 = mybir.dt.float32
    fp16 = mybir.dt.float16

    nc = bacc.Bacc()
    qkv = nc.dram_tensor("qkv", [NCHUNK, 128, IN_COLS], fp16, kind="ExternalInput")
    out = nc.dram_tensor("out", [NCHUNK, 128, OUT_COLS], fp16, kind="ExternalOutput")

    with TileContext(nc) as tc:
        PREFETCH = 9

        with (
            tc.tile_pool(name="qkv_in", bufs=8) as in_pool,
            tc.tile_pool(name="o_out", bufs=6) as o_pool,
            tc.tile_pool(name="exp", bufs=3) as exp_pool,
            tc.tile_pool(name="psl", bufs=2, space="PSUM") as psl_pool,
            tc.tile_pool(name="pso", bufs=6, space="PSUM") as pso_pool,
        ):
            in_tiles = []
            HC = IN_COLS // 2  # 3080

            def issue_in(n):
                in_t = in_pool.tile([128, IN_COLS], fp16, tag="qkv")
                if n == 0:
                    # ramp compression: chunk 0 is host-packed as two
                    # [q|k|v] half-blocks so QK can start on groups 0-7
                    # as soon as the first half lands
                    nc.sync.dma_start(out=in_t[:, :HC], in_=qkv[0, :, :HC])
                    nc.sync.dma_start(out=in_t[:, HC:], in_=qkv[0, :, HC:])
                else:
                    nc.sync.dma_start(out=in_t, in_=qkv[n])
                in_tiles.append(in_t)

            def offs(n, g):
                # (q, k, v) column offsets for group g of chunk n
                if n == 0:
                    hb, gg = (g // 8) * HC, g % 8
                    return (hb + gg * D, hb + 8 * D + gg * D,
                            hb + 16 * D + gg * VCOL)
                return (g * D, QK_COLS + g * D, V_OFF + g * VCOL)

            for n in range(min(PREFETCH, NCHUNK)):
                issue_in(n)

            def issue_qk(n):
                """Chunk n's logits into one full [128,512] PSUM bank."""
                in_t = in_tiles[n]
                psl = psl_pool.tile([128, 512], fp32, tag="psl")
                for g in range(NG):
                    q4, t = g >> 2, g & 3
                    qo, ko, _ = offs(n, g)
                    for j in range(GP):
                        c = slice(qo + 32 * j, qo + 32 * j + 32)
                        ck = slice(ko + 32 * j, ko + 32 * j + 32)
                        nc.tensor.matmul(
                            psl[32 * j:32 * j + 32,
                                q4 * 128 + 32 * t:q4 * 128 + 32 * t + 32],
                            in_t[:, ck],       # stationary: k of (g, j)
                            in_t[:, c],        # moving:     q of (g, j)
                            start=True, stop=True,
                            tile_position=(0, 32 * j),
                        )
                return psl

            psl_n = issue_qk(0)
            for n in range(NCHUNK):
                # --- one exp per chunk ---
                exp_sb = exp_pool.tile([128, 512], fp16, tag="exp_sb")
                nc.scalar.activation(
                    exp_sb, psl_n, mybir.ActivationFunctionType.Exp, scale=_SCALE,
                )
                if n + 1 < NCHUNK:
                    psl_n = issue_qk(n + 1)

                # --- WV + denominator, evacuate unnormalized ---
                in_t = in_tiles[n]
                out_t = o_pool.tile([128, OUT_COLS], fp16, tag="out")
                for pair in range(NG // 2):
                    psum_o = pso_pool.tile([128, 2 * VCOL], fp32, tag="pso")
                    for u in range(2):
                        g = 2 * pair + u
                        q4, t = g >> 2, g & 3
                        for j in range(GP):
                            r = slice(32 * j, 32 * j + 32)
                            vo = offs(n, g)[2]
                            nc.tensor.matmul(
                                psum_o[r, u * VCOL:(u + 1) * VCOL],
                                exp_sb[r, q4 * 128 + 32 * t:q4 * 128 + 32 * t + 32],
                                in_t[r, vo:vo + VCOL],
                                start=True, stop=True,
                                tile_position=(32 * j, 32 * j),
                            )
                    dst = out_t[:, pair * 2 * VCOL:(pair + 1) * 2 * VCOL]
                    if pair < 2:
                        nc.scalar.copy(dst, psum_o)
                    else:
                        nc.vector.tensor_copy(dst, psum_o)
                    # tail compression: the last two chunks drain their first
                    # half early so the final output DMA overlaps the
                    # remaining copies instead of serializing after them
                    if n >= NCHUNK - 2 and pair == 3:
                        half = OUT_COLS // 2
                        nc.sync.dma_start(out=out[n, :, :half],
                                          in_=out_t[:, :half])

                # output trigger + next prefetch on the Sync ring
                if n >= NCHUNK - 2:
                    half = OUT_COLS // 2
                    nc.sync.dma_start(out=out[n, :, half:], in_=out_t[:, half:])
                else:
                    nc.sync.dma_start(out=out[n], in_=out_t)
                if n + PREFETCH < NCHUNK:
                    issue_in(n + PREFETCH)

    nc.compile()
    return nc


def _host_pack(q, k, v):
    """Build per-core device input arrays from full fp32 inputs."""
    qf = np.ascontiguousarray(q, dtype=np.float32).reshape(POS, H, D)
    kf = np.ascontiguousarray(k, dtype=np.float32).reshape(POS, H, D)
    vf = np.ascontiguousarray(v, dtype=np.float32).reshape(POS, H, D)

    nchunk_tot = POS // CHUNK_POS
    # q,k: [chunk, group, i, h, d] -> [chunk, d, (group, i, h)]
    def to_qt(x):
        x = x.reshape(nchunk_tot, NG, GP, H, D)
        x = x.transpose(0, 4, 1, 2, 3)
        return np.ascontiguousarray(x.reshape(nchunk_tot, D, NG * GP * H)).astype(np.float16)

    qt_all = to_qt(qf)
    kt_all = to_qt(kf)

    # v: [chunk, group, i, gh, d] -> [chunk, (i,gh), (group, d|1)]
    vv = vf.reshape(nchunk_tot, NG, GP, H, D).transpose(0, 2, 3, 1, 4)
    vp_all = np.ones((nchunk_tot, GP, H, NG, VCOL), dtype=np.float32)
    vp_all[..., :D] = vv
    vp_all = vp_all.reshape(nchunk_tot, GP * H, NG * VCOL).astype(np.float16)

    qkv_all = np.concatenate([qt_all, kt_all, vp_all], axis=2)
    qkv_all = np.ascontiguousarray(qkv_all)
    # each core's FIRST chunk: [q0-7|k0-7|v0-7 | q8-15|k8-15|v8-15]
    for n0 in range(0, qkv_all.shape[0], NCHUNK):
        row = qkv_all[n0]
        q, k, vv2 = row[:, :QK_COLS], row[:, QK_COLS:2 * QK_COLS], row[:, 2 * QK_COLS:]
        hq, hk, hv = QK_COLS // 2, QK_COLS // 2, NG * VCOL // 2
        qkv_all[n0] = np.concatenate(
            [q[:, :hq], k[:, :hk], vv2[:, :hv],
             q[:, hq:], k[:, hk:], vv2[:, hv:]], axis=1)

    in_maps = []
    for c in range(NCORES):
        sl = slice(c * NCHUNK, (c + 1) * NCHUNK)
        in_maps.append({"qkv": np.ascontiguousarray(qkv_all[sl])})
    return in_maps


def _host_unpack(outs):
    """Per-core [NCHUNK, 128, NG*VCOL] fp16 -> full [B, S, H*D] fp32."""
    full = np.concatenate(outs, axis=0).astype(np.float32)
    nchunk_tot = POS // CHUNK_POS
    full = full.reshape(nchunk_tot, GP, H, NG, VCOL)  # [chunk, i, h, g, d|z]
    num = full[..., :D]
    den = full[..., D:D + 1]
    res = num / den
    res = res.transpose(0, 3, 1, 2, 4)                # [chunk, g, i, h, d]
    return np.ascontiguousarray(res.reshape(B, S, H * D), dtype=np.float32)


def kernel(q, k, v, _trace=False):
    global _program
    from concourse.bass_utils import run_bass_kernel_spmd

    if _program is None:
        _program = _build_program()

    in_maps = _host_pack(q, k, v)
    res = run_bass_kernel_spmd(_program, in_maps, list(range(NCORES)), trace=_trace)
    outs = [res.results[c]["out"] for c in range(NCORES)]
    result = _host_unpack(outs)
    if _trace:
        return result, res
    return result
